# revision 29
# baseline (speedup 1.0000x reference)
"""RBF-kernel causal attention on 8 Trainium2 NeuronCores.

B=2, H=16, N=2048, D=64. Shards the 32 (b,h) attention instances across 8
cores (4 heads per core). Math notes:

  logits = -relu(||q-k||^2)/sqrt(D); relu is a no-op (||q-k||^2 >= 0 up to
  rounding), and softmax is invariant to per-query offsets, so
      softmax_n(-(qsq_m + ksq_n - 2 qk)/8) == softmax_n(qk/4 - ksq_n/8)
  We compute P'' = exp(0.25 * K Q^T) in a [key, query] layout and fold the
  exp(-0.125 ksq_n) per-key factor into V (and into the appended ones-column
  that produces the softmax denominator):
      [O^T | l] accumulates via matmul(lhsT=V_aug_scaled, rhs=P'').
  Final output O[m,d] = OT[d,m] / l[m], un-transposed via PE transpose.

Emission is manually software-pipelined: head h+1's setup chunks (transposes,
ksq, V scaling) are interleaved between head h's query blocks so the tile
scheduler (limited lookahead) can overlap them.
"""

import sys

if "/opt/trn_rl_repo" not in sys.path:
    sys.path.insert(0, "/opt/trn_rl_repo")

import numpy as np

import concourse.bacc as bacc
import concourse.mybir as mybir
import concourse.tile as tile
from concourse.masks import make_identity

B, H, N, D = 2, 16, 2048, 64
NCORES = 8
HPC = (B * H) // NCORES  # heads per core = 4
P = 128                  # partitions
NT = N // P              # key tiles per head = 16
QB = 512                 # query block (matmul moving dim)
MBS = N // QB            # query blocks per head = 4
G = 2                    # key tiles per exp/ACT group (2 PSUM banks)

F32 = mybir.dt.float32
# float32r = relaxed-precision fp32 matmul (1 cycle/row at moving dim >= 256
# instead of 4 for float32)
MM_DT = mybir.dt.float32r


def build_nc():
    nc = bacc.Bacc("TRN2", target_bir_lowering=False, debug=False)
    q = nc.dram_tensor("q", [HPC, N, D], F32, kind="ExternalInput")
    k = nc.dram_tensor("k", [HPC, N, D], F32, kind="ExternalInput")
    v = nc.dram_tensor("v", [HPC, N, D], F32, kind="ExternalInput")
    out = nc.dram_tensor("out", [HPC, N, D], F32, kind="ExternalOutput")

    with tile.TileContext(nc) as tc:
        with (
            tc.tile_pool(name="const", bufs=1) as const_pool,
            tc.tile_pool(name="loads", bufs=1) as load_pool,
            tc.tile_pool(name="head", bufs=2) as head_pool,
            tc.tile_pool(name="work", bufs=3) as work_pool,
            tc.tile_pool(name="p", bufs=4) as p_pool,
            tc.tile_pool(name="epi", bufs=3) as epi_pool,
            tc.tile_pool(name="st", bufs=3, space="PSUM") as st_pool,
            tc.tile_pool(name="otp", bufs=2, space="PSUM") as ot_pool,
        ):
            identity = const_pool.tile([P, P], F32)
            make_identity(nc, identity)

            # prefetch every head's inputs up front: no-wait DMAs stream in
            # the background while compute proceeds
            knats, qnats, vtmps = [], [], []
            for h in range(HPC):
                # quarter-granular loads so the first transposes start as
                # soon as the first chunk lands, not after the whole head
                knat = load_pool.tile([P, NT, D], F32, tag=f"knat{h}")
                # q loaded DOUBLED along a repeat dim (two passes over DRAM):
                # transposing [128m, (2,64d)] then yields Q^T duplicated on
                # both partition halves, as the row-packed matmuls need
                qnat = load_pool.tile([P, NT, 2, D], F32, tag=f"qnat{h}")
                vtmp = load_pool.tile([P, NT, D], F32, tag=f"vtmp{h}")
                kq = k[h].rearrange("(t p) d -> p t d", p=P)
                qq = q[h].rearrange("(t p) d -> p t d", p=P)
                vq = v[h].rearrange("(t p) d -> p t d", p=P)
                nch = 4 if h == 0 else 1
                w_ = NT // nch
                for c in range(nch):
                    ts = slice(w_ * c, w_ * c + w_)
                    nc.sync.dma_start(knat[:, ts, :], kq[:, ts, :])
                    for r in range(2):
                        nc.sync.dma_start(qnat[:, ts, r, :], qq[:, ts, :])
                    nc.sync.dma_start(vtmp[:, ts, :], vq[:, ts, :])
                knats.append(knat)
                qnats.append(qnat)
                vtmps.append(vtmp)

            heads = [{} for _ in range(HPC)]

            def setup_chunks(h):
                """Emission chunks for head h's setup, in dependency order."""
                st = heads[h]

                def allocs():
                    st["ksq"] = head_pool.tile([P, NT], F32, tag="ksq", name="ksq")
                    st["w"] = head_pool.tile([P, NT], F32, tag="w", name="w")
                    st["vaug"] = head_pool.tile(
                        [P, NT, D + 1], MM_DT, tag="vaug", name="vaug"
                    )
                    # kt: key-tile PAIRS stacked on partition halves
                    # (even tile at partitions 0:64, odd at 64:128) so two
                    # QK matmuls can row-pack the PE array concurrently.
                    st["kt"] = head_pool.tile(
                        [P, NT // 2, P], MM_DT, tag="kt", name="kt"
                    )
                    # qt: Q^T duplicated into both partition halves (the
                    # row-packed matmuls stream rhs partitions 0:64 and
                    # 64:128 into array row groups 0-1 and 2-3)
                    st["qt"] = head_pool.tile([P, NT, P], MM_DT, tag="qt", name="qt")

                def scale_chunk(c):
                    # per-quarter V_aug build: runs as soon as that quarter
                    # of k and v has landed
                    def run():
                        ts = slice(4 * c, 4 * c + 4)
                        knat, vtmp = knats[h], vtmps[h]
                        ksq, w, vaug = st["ksq"], st["w"], st["vaug"]
                        ktmp = work_pool.tile([P, 4, D], F32, tag="ktmp")
                        nc.vector.tensor_mul(
                            out=ktmp[:], in0=knat[:, ts, :], in1=knat[:, ts, :]
                        )
                        nc.vector.tensor_reduce(
                            ksq[:, ts], ktmp[:],
                            axis=mybir.AxisListType.X, op=mybir.AluOpType.add,
                        )
                        nc.scalar.activation(
                            w[:, ts], ksq[:, ts],
                            mybir.ActivationFunctionType.Exp, scale=-0.125,
                        )
                        nc.vector.tensor_mul(
                            out=vaug[:, ts, :D],
                            in0=vtmp[:, ts, :],
                            in1=w[:, ts, None].to_broadcast((P, 4, D)),
                        )
                        nc.vector.tensor_copy(
                            out=vaug[:, ts, D : D + 1], in_=w[:, ts, None]
                        )

                    return run

                def ktr_group(g):
                    # 4 pair-transposes: [128n, (2t, 64d)] -> [(2t, 64d), 128n]
                    # lands even tile at partitions 0:64, odd at 64:128
                    def run():
                        src = knats[h]
                        dst = heads[h]["kt"]
                        tp = st_pool.tile([P, 4, P], F32, tag="stg", name="tp")
                        for j in range(4):
                            pr = 4 * g + j
                            nc.tensor.transpose(
                                tp[:, j, :], src[:, 2 * pr : 2 * pr + 2, :],
                                identity[:],
                            )
                        nc.vector.tensor_copy(
                            out=dst[:, 4 * g : 4 * g + 4, :], in_=tp[:]
                        )

                    return run

                def qtr_group(g):
                    # transpose a 0-stride doubled view [128m, (2, 64d)] so
                    # the output holds Q^T duplicated on both partition
                    # halves (rows 0:64 and 64:128) in one shot
                    def run():
                        src = qnats[h]
                        dst = heads[h]["qt"]
                        tp = st_pool.tile([P, 4, P], F32, tag="stg", name="tp")
                        for j in range(4):
                            nc.tensor.transpose(
                                tp[:, j, :], src[:, 4 * g + j, :, :], identity[:]
                            )
                        nc.vector.tensor_copy(
                            out=dst[:, 4 * g : 4 * g + 4, :], in_=tp[:]
                        )

                    return run

                # query block mb needs kt pair-groups up to (2mb+1)//4, qt
                # group mb, and vaug quarter mb; yield in dependency order
                yield allocs
                yield scale_chunk(0)
                yield ktr_group(0)
                yield qtr_group(0)
                yield scale_chunk(1)
                yield qtr_group(1)
                yield ktr_group(1)
                yield scale_chunk(2)
                yield qtr_group(2)
                yield scale_chunk(3)
                yield qtr_group(3)

            def job_chunks(h, mb):
                """Chunks of one (head, query-block) job, for interleaving."""
                kt, qt, vaug = heads[h]["kt"], heads[h]["qt"], heads[h]["vaug"]
                nsub = 4 * mb          # sub-diagonal key tiles
                qt_lo = qt[:D, 4 * mb : 4 * mb + 4, :]   # [64, 512]
                qt_hi = qt[D:, 4 * mb : 4 * mb + 4, :]   # [64, 512]
                ntiles = nsub + 4
                jst = {"prev": None, "ot": None}

                def sub_group(s):
                    def run():
                        if jst["ot"] is None:
                            jst["ot"] = ot_pool.tile(
                                [D + 1, QB], F32, tag="ot", name="ot"
                            )
                        stg = st_pool.tile([P, G, QB], F32, tag="stg")
                        pr = s // 2
                        nc.tensor.matmul(
                            stg[:, 0, :], kt[:D, pr, :], qt_lo,
                            start=True, stop=True, skip_group_check=True,
                        )
                        nc.tensor.matmul(
                            stg[:, 1, :], kt[D:, pr, :], qt_hi,
                            start=True, stop=True, skip_group_check=True,
                        )
                        pg = p_pool.tile([P, G, QB], MM_DT, tag="pg")
                        nc.scalar.activation(
                            pg[:], stg[:],
                            mybir.ActivationFunctionType.Exp, scale=0.25,
                        )
                        if jst["prev"] is not None:
                            _emit_pv(nc, jst["ot"], vaug, jst["prev"], ntiles)
                        jst["prev"] = (pg, [s, s + 1])

                    return run

                def diag_group(a):
                    def run():
                        if jst["ot"] is None:
                            jst["ot"] = ot_pool.tile(
                                [D + 1, QB], F32, tag="ot", name="ot"
                            )
                        if a == 0:
                            jst["pgd"] = p_pool.tile([P, 4, QB], MM_DT, tag="pgd", name="pgd")
                        pgd = jst["pgd"]
                        # columns m < 128*(2a) of tiles (2a, 2a+1) are fully
                        # masked: skip their QK matmul + exp; affine_select
                        # below zero-fills that (otherwise garbage) region.
                        c0 = P * 2 * a
                        stg = st_pool.tile([P, G, QB], F32, tag="stg")
                        pr = 2 * mb + a
                        nc.tensor.matmul(
                            stg[:, 0, c0:],
                            kt[:D, pr, :],
                            qt[:D, 4 * mb + 2 * a : 4 * mb + 4, :],
                            start=True, stop=True, skip_group_check=True,
                        )
                        nc.tensor.matmul(
                            stg[:, 1, c0:],
                            kt[D:, pr, :],
                            qt[D:, 4 * mb + 2 * a : 4 * mb + 4, :],
                            start=True, stop=True, skip_group_check=True,
                        )
                        nc.scalar.activation(
                            pgd[:, 2 * a : 2 * a + 2, c0:], stg[:, :, c0:],
                            mybir.ActivationFunctionType.Exp, scale=0.25,
                        )
                        for j in range(G):
                            # keep pgd[n, jj, m] iff m - n - 128 jj >= 0
                            jj = 2 * a + j
                            nc.gpsimd.affine_select(
                                out=pgd[:, jj, :], in_=pgd[:, jj, :],
                                compare_op=mybir.AluOpType.is_ge, fill=0.0,
                                base=-P * jj, pattern=[[1, QB]],
                                channel_multiplier=-1,
                            )

                    return run

                def pv_epilogue():
                    ot, pgd = jst["ot"], jst["pgd"]
                    if jst["prev"] is not None:
                        _emit_pv(nc, ot, vaug, jst["prev"], ntiles)
                    for j in range(4):
                        nc.tensor.matmul(
                            ot[:], vaug[:, 4 * mb + j, :], pgd[:, j, :],
                            start=(nsub == 0 and j == 0), stop=(j == 3),
                            skip_group_check=True,
                        )
                    # epilogue: transpose + normalize + store
                    ot_sb = epi_pool.tile([D + 1, QB], F32, tag="ot_sb")
                    nc.vector.tensor_copy(out=ot_sb[:], in_=ot[:])
                    tpo = ot_pool.tile([P, 4, D + 1], F32, tag="ot", name="tpo")
                    for j in range(4):
                        nc.tensor.transpose(
                            tpo[:, j, :],
                            ot_sb[:, j * P : (j + 1) * P],
                            identity[: D + 1, : D + 1],
                        )
                    linv = epi_pool.tile([P, 4], F32, tag="linv")
                    nc.vector.reciprocal(linv[:], tpo[:, :, D])
                    o_sb = epi_pool.tile([P, 4, D], F32, tag="o_sb")
                    for j in range(4):
                        nc.vector.tensor_scalar_mul(
                            o_sb[:, j, :], tpo[:, j, :D], linv[:, j : j + 1]
                        )
                    nc.sync.dma_start(
                        out[h, mb * QB : (mb + 1) * QB, :].rearrange(
                            "(j p) d -> p j d", p=P
                        ),
                        o_sb[:],
                    )

                chunks = [sub_group(s) for s in range(0, nsub, G)]
                chunks += [diag_group(0), diag_group(1), pv_epilogue]
                return chunks

            # ---- software-pipelined emission: depth-2 job interleave ----
            for c in setup_chunks(0):
                c()
            pending = []          # next head's setup chunks, dripped in
            jobs = [(h, mb) for h in range(HPC) for mb in range(MBS)]
            active = []           # up to 2 jobs' chunk queues
            ji = 0
            drip = 0
            while active or ji < len(jobs):
                while len(active) < 2 and ji < len(jobs):
                    h, mb = jobs[ji]
                    if mb == 0 and pending:
                        # head h's setup must be fully emitted before its
                        # first job
                        for c in pending:
                            c()
                        pending = []
                    if mb == 0 and h + 1 < HPC:
                        pending = list(setup_chunks(h + 1))
                    active.append(job_chunks(h, mb))
                    ji += 1
                for q_ in list(active):
                    q_.pop(0)()
                    drip += 1
                    if drip % 3 == 0 and pending:
                        pending.pop(0)()
                active = [q_ for q_ in active if q_]
            for c in pending:
                c()

    nc.compile()
    return nc


def _emit_pv(nc, ot, vaug, group, ntiles):
    pg, tiles = group
    for j, nt in enumerate(tiles):
        nc.tensor.matmul(
            ot[:],
            vaug[:, nt, :],
            pg[:, j, :],
            start=(nt == 0),
            stop=(nt == ntiles - 1),
            skip_group_check=True,
        )


_NC = None


def _get_nc():
    global _NC
    if _NC is None:
        _NC = build_nc()
    return _NC


def kernel(q: np.ndarray, k: np.ndarray, v: np.ndarray) -> np.ndarray:
    from concourse.bass_utils import run_bass_kernel_spmd

    nc = _get_nc()
    qf = np.ascontiguousarray(np.asarray(q, dtype=np.float32).reshape(B * H, N, D))
    kf = np.ascontiguousarray(np.asarray(k, dtype=np.float32).reshape(B * H, N, D))
    vf = np.ascontiguousarray(np.asarray(v, dtype=np.float32).reshape(B * H, N, D))
    in_maps = [
        {
            "q": np.ascontiguousarray(qf[c * HPC : (c + 1) * HPC]),
            "k": np.ascontiguousarray(kf[c * HPC : (c + 1) * HPC]),
            "v": np.ascontiguousarray(vf[c * HPC : (c + 1) * HPC]),
        }
        for c in range(NCORES)
    ]
    res = run_bass_kernel_spmd(nc, in_maps, core_ids=list(range(NCORES)))
    outs = [res.results[c]["out"] for c in range(NCORES)]
    return np.concatenate(outs, axis=0).reshape(B, H, N, D)


if __name__ == "__main__":
    rng = np.random.default_rng(0)
    qq = rng.standard_normal((B, H, N, D), dtype=np.float32)
    kk = rng.standard_normal((B, H, N, D), dtype=np.float32)
    vv = rng.standard_normal((B, H, N, D), dtype=np.float32)
    o = kernel(q=qq, k=kk, v=vv)
    print("kernel ran, out shape", o.shape, "finite:", np.isfinite(o).all())


# revision 31
# speedup vs baseline: 1.0588x; 1.0588x over previous
"""RBF-kernel causal attention on 8 Trainium2 NeuronCores.

B=2, H=16, N=2048, D=64. Shards the 32 (b,h) attention instances across 8
cores (4 heads per core). Math notes:

  logits = -relu(||q-k||^2)/sqrt(D); relu is a no-op (||q-k||^2 >= 0 up to
  rounding), and softmax is invariant to per-query offsets, so
      softmax_n(-(qsq_m + ksq_n - 2 qk)/8) == softmax_n(qk/4 - ksq_n/8)
  We compute P'' = exp(0.25 * K Q^T) in a [key, query] layout and fold the
  exp(-0.125 ksq_n) per-key factor into V (and into the appended ones-column
  that produces the softmax denominator):
      [O^T | l] accumulates via matmul(lhsT=V_aug_scaled, rhs=P'').
  Final output O[m,d] = OT[d,m] / l[m], un-transposed via PE transpose.

Emission is manually software-pipelined: head h+1's setup chunks (transposes,
ksq, V scaling) are interleaved between head h's query blocks so the tile
scheduler (limited lookahead) can overlap them.
"""

import sys

if "/opt/trn_rl_repo" not in sys.path:
    sys.path.insert(0, "/opt/trn_rl_repo")

import numpy as np

import concourse.bacc as bacc
import concourse.mybir as mybir
import concourse.tile as tile
from concourse.masks import make_identity

B, H, N, D = 2, 16, 2048, 64
NCORES = 8
HPC = (B * H) // NCORES  # heads per core = 4
P = 128                  # partitions
NT = N // P              # key tiles per head = 16
QB = 512                 # query block (matmul moving dim)
MBS = N // QB            # query blocks per head = 4
G = 2                    # key tiles per exp/ACT group (2 PSUM banks)

F32 = mybir.dt.float32
# float32r = relaxed-precision fp32 matmul (1 cycle/row at moving dim >= 256
# instead of 4 for float32)
MM_DT = mybir.dt.float32r


def build_nc():
    nc = bacc.Bacc("TRN2", target_bir_lowering=False, debug=False)
    q = nc.dram_tensor("q", [HPC, N, D], F32, kind="ExternalInput")
    k = nc.dram_tensor("k", [HPC, N, D], F32, kind="ExternalInput")
    v = nc.dram_tensor("v", [HPC, N, D], F32, kind="ExternalInput")
    out = nc.dram_tensor("out", [HPC, N, D], F32, kind="ExternalOutput")

    with tile.TileContext(nc) as tc:
        with (
            tc.tile_pool(name="const", bufs=1) as const_pool,
            tc.tile_pool(name="loads", bufs=1) as load_pool,
            tc.tile_pool(name="head", bufs=2) as head_pool,
            tc.tile_pool(name="work", bufs=3) as work_pool,
            tc.tile_pool(name="p", bufs=4) as p_pool,
            tc.tile_pool(name="epi", bufs=3) as epi_pool,
            tc.tile_pool(name="st", bufs=3, space="PSUM") as st_pool,
            tc.tile_pool(name="otp", bufs=2, space="PSUM") as ot_pool,
        ):
            identity = const_pool.tile([P, P], F32)
            make_identity(nc, identity)

            # prefetch every head's inputs up front: no-wait DMAs stream in
            # the background while compute proceeds
            knats, qnats, vtmps = [], [], []
            for h in range(HPC):
                # quarter-granular loads so the first transposes start as
                # soon as the first chunk lands, not after the whole head
                knat = load_pool.tile([P, NT, D], F32, tag=f"knat{h}")
                # q loaded DOUBLED along a repeat dim (two passes over DRAM):
                # transposing [128m, (2,64d)] then yields Q^T duplicated on
                # both partition halves, as the row-packed matmuls need
                qnat = load_pool.tile([P, NT, 2, D], F32, tag=f"qnat{h}")
                vtmp = load_pool.tile([P, NT, D], F32, tag=f"vtmp{h}")
                kq = k[h].rearrange("(t p) d -> p t d", p=P)
                qq = q[h].rearrange("(t p) d -> p t d", p=P)
                vq = v[h].rearrange("(t p) d -> p t d", p=P)
                nch = 4 if h == 0 else 1
                w_ = NT // nch
                for c in range(nch):
                    ts = slice(w_ * c, w_ * c + w_)
                    nc.sync.dma_start(knat[:, ts, :], kq[:, ts, :])
                    for r in range(2):
                        nc.sync.dma_start(qnat[:, ts, r, :], qq[:, ts, :])
                    nc.sync.dma_start(vtmp[:, ts, :], vq[:, ts, :])
                knats.append(knat)
                qnats.append(qnat)
                vtmps.append(vtmp)

            heads = [{} for _ in range(HPC)]

            def setup_chunks(h):
                """Emission chunks for head h's setup, in dependency order."""
                st = heads[h]

                def allocs():
                    st["ksq"] = head_pool.tile([P, NT], F32, tag="ksq", name="ksq")
                    st["w"] = head_pool.tile([P, NT], F32, tag="w", name="w")
                    st["vaug"] = head_pool.tile(
                        [P, NT, D + 1], MM_DT, tag="vaug", name="vaug"
                    )
                    # kt: key-tile PAIRS stacked on partition halves
                    # (even tile at partitions 0:64, odd at 64:128) so two
                    # QK matmuls can row-pack the PE array concurrently.
                    st["kt"] = head_pool.tile(
                        [P, NT // 2, P], MM_DT, tag="kt", name="kt"
                    )
                    # qt: Q^T duplicated into both partition halves (the
                    # row-packed matmuls stream rhs partitions 0:64 and
                    # 64:128 into array row groups 0-1 and 2-3)
                    st["qt"] = head_pool.tile([P, NT, P], MM_DT, tag="qt", name="qt")

                def scale_chunk(c):
                    # per-quarter V_aug build: runs as soon as that quarter
                    # of k and v has landed
                    def run():
                        ts = slice(4 * c, 4 * c + 4)
                        knat, vtmp = knats[h], vtmps[h]
                        ksq, w, vaug = st["ksq"], st["w"], st["vaug"]
                        ktmp = work_pool.tile([P, 4, D], F32, tag="ktmp")
                        nc.vector.tensor_mul(
                            out=ktmp[:], in0=knat[:, ts, :], in1=knat[:, ts, :]
                        )
                        nc.vector.tensor_reduce(
                            ksq[:, ts], ktmp[:],
                            axis=mybir.AxisListType.X, op=mybir.AluOpType.add,
                        )
                        nc.scalar.activation(
                            w[:, ts], ksq[:, ts],
                            mybir.ActivationFunctionType.Exp, scale=-0.125,
                        )
                        nc.vector.tensor_mul(
                            out=vaug[:, ts, :D],
                            in0=vtmp[:, ts, :],
                            in1=w[:, ts, None].to_broadcast((P, 4, D)),
                        )
                        nc.vector.tensor_copy(
                            out=vaug[:, ts, D : D + 1], in_=w[:, ts, None]
                        )

                    return run

                def ktr_group(g):
                    # 4 pair-transposes: [128n, (2t, 64d)] -> [(2t, 64d), 128n]
                    # lands even tile at partitions 0:64, odd at 64:128
                    def run():
                        src = knats[h]
                        dst = heads[h]["kt"]
                        tp = st_pool.tile([P, 4, P], F32, tag="stg", name="tp")
                        for j in range(4):
                            pr = 4 * g + j
                            nc.tensor.transpose(
                                tp[:, j, :], src[:, 2 * pr : 2 * pr + 2, :],
                                identity[:],
                            )
                        nc.vector.tensor_copy(
                            out=dst[:, 4 * g : 4 * g + 4, :], in_=tp[:]
                        )

                    return run

                def qtr_group(g):
                    # transpose a 0-stride doubled view [128m, (2, 64d)] so
                    # the output holds Q^T duplicated on both partition
                    # halves (rows 0:64 and 64:128) in one shot
                    def run():
                        src = qnats[h]
                        dst = heads[h]["qt"]
                        tp = st_pool.tile([P, 4, P], F32, tag="stg", name="tp")
                        for j in range(4):
                            nc.tensor.transpose(
                                tp[:, j, :], src[:, 4 * g + j, :, :], identity[:]
                            )
                        nc.vector.tensor_copy(
                            out=dst[:, 4 * g : 4 * g + 4, :], in_=tp[:]
                        )

                    return run

                # query block mb needs kt pair-groups up to (2mb+1)//4, qt
                # group mb, and vaug quarter mb; yield in dependency order
                yield allocs
                yield scale_chunk(0)
                yield ktr_group(0)
                yield qtr_group(0)
                yield scale_chunk(1)
                yield qtr_group(1)
                yield ktr_group(1)
                yield scale_chunk(2)
                yield qtr_group(2)
                yield scale_chunk(3)
                yield qtr_group(3)

            def job_chunks(h, mb):
                """Chunks of one (head, query-block) job, for interleaving."""
                kt, qt, vaug = heads[h]["kt"], heads[h]["qt"], heads[h]["vaug"]
                nsub = 4 * mb          # sub-diagonal key tiles
                qt_lo = qt[:D, 4 * mb : 4 * mb + 4, :]   # [64, 512]
                qt_hi = qt[D:, 4 * mb : 4 * mb + 4, :]   # [64, 512]
                ntiles = nsub + 4
                jst = {"prev": None, "ot": None}

                def sub_group(s):
                    def run():
                        if jst["ot"] is None:
                            jst["ot"] = ot_pool.tile(
                                [D + 1, QB], F32, tag="ot", name="ot"
                            )
                        stg = st_pool.tile([P, G, QB], F32, tag="stg")
                        pr = s // 2
                        nc.tensor.matmul(
                            stg[:, 0, :], kt[:D, pr, :], qt_lo,
                            start=True, stop=True, skip_group_check=True,
                        )
                        nc.tensor.matmul(
                            stg[:, 1, :], kt[D:, pr, :], qt_hi,
                            start=True, stop=True, skip_group_check=True,
                        )
                        pg = p_pool.tile([P, G, QB], MM_DT, tag="pg")
                        nc.scalar.activation(
                            pg[:], stg[:],
                            mybir.ActivationFunctionType.Exp, scale=0.25,
                        )
                        if jst["prev"] is not None:
                            _emit_pv(nc, jst["ot"], vaug, jst["prev"], ntiles)
                        jst["prev"] = (pg, [s, s + 1])

                    return run

                def diag_group(a):
                    def run():
                        if jst["ot"] is None:
                            jst["ot"] = ot_pool.tile(
                                [D + 1, QB], F32, tag="ot", name="ot"
                            )
                        if a == 0:
                            jst["pgd"] = p_pool.tile([P, 4, QB], MM_DT, tag="pgd", name="pgd")
                        pgd = jst["pgd"]
                        # columns m < 128*(2a) of tiles (2a, 2a+1) are fully
                        # masked: skip their QK matmul + exp; affine_select
                        # below zero-fills that (otherwise garbage) region.
                        c0 = P * 2 * a
                        stg = st_pool.tile([P, G, QB], F32, tag="stg")
                        pr = 2 * mb + a
                        nc.tensor.matmul(
                            stg[:, 0, c0:],
                            kt[:D, pr, :],
                            qt[:D, 4 * mb + 2 * a : 4 * mb + 4, :],
                            start=True, stop=True, skip_group_check=True,
                        )
                        nc.tensor.matmul(
                            stg[:, 1, c0:],
                            kt[D:, pr, :],
                            qt[D:, 4 * mb + 2 * a : 4 * mb + 4, :],
                            start=True, stop=True, skip_group_check=True,
                        )
                        nc.scalar.activation(
                            pgd[:, 2 * a : 2 * a + 2, c0:], stg[:, :, c0:],
                            mybir.ActivationFunctionType.Exp, scale=0.25,
                        )
                        for j in range(G):
                            # keep pgd[n, jj, m] iff m - n - 128 jj >= 0
                            jj = 2 * a + j
                            nc.gpsimd.affine_select(
                                out=pgd[:, jj, :], in_=pgd[:, jj, :],
                                compare_op=mybir.AluOpType.is_ge, fill=0.0,
                                base=-P * jj, pattern=[[1, QB]],
                                channel_multiplier=-1,
                            )

                    return run

                def pv_epilogue():
                    ot, pgd = jst["ot"], jst["pgd"]
                    if jst["prev"] is not None:
                        _emit_pv(nc, ot, vaug, jst["prev"], ntiles)
                    for j in range(4):
                        nc.tensor.matmul(
                            ot[:], vaug[:, 4 * mb + j, :], pgd[:, j, :],
                            start=(nsub == 0 and j == 0), stop=(j == 3),
                            skip_group_check=True,
                        )
                    # epilogue: transpose + normalize + store
                    ot_sb = epi_pool.tile([D + 1, QB], F32, tag="ot_sb")
                    nc.vector.tensor_copy(out=ot_sb[:], in_=ot[:])
                    tpo = ot_pool.tile([P, 4, D + 1], F32, tag="ot", name="tpo")
                    for j in range(4):
                        nc.tensor.transpose(
                            tpo[:, j, :],
                            ot_sb[:, j * P : (j + 1) * P],
                            identity[: D + 1, : D + 1],
                        )
                    linv = epi_pool.tile([P, 4], F32, tag="linv")
                    nc.vector.reciprocal(linv[:], tpo[:, :, D])
                    o_sb = epi_pool.tile([P, 4, D], F32, tag="o_sb")
                    for j in range(4):
                        nc.vector.tensor_scalar_mul(
                            o_sb[:, j, :], tpo[:, j, :D], linv[:, j : j + 1]
                        )
                    nc.sync.dma_start(
                        out[h, mb * QB : (mb + 1) * QB, :].rearrange(
                            "(j p) d -> p j d", p=P
                        ),
                        o_sb[:],
                    )

                chunks = [sub_group(s) for s in range(0, nsub, G)]
                chunks += [diag_group(0), diag_group(1), pv_epilogue]
                return chunks

            # ---- software-pipelined emission: depth-2 job interleave ----
            for c in setup_chunks(0):
                c()
            pending = []          # next head's setup chunks, dripped in
            jobs = [(h, mb) for h in range(HPC) for mb in range(MBS)]
            active = []           # up to 2 jobs' chunk queues
            ji = 0
            drip = 0
            while active or ji < len(jobs):
                while len(active) < 2 and ji < len(jobs):
                    h, mb = jobs[ji]
                    if mb == 0 and pending:
                        # head h's setup must be fully emitted before its
                        # first job
                        for c in pending:
                            c()
                        pending = []
                    if mb == 0 and h + 1 < HPC:
                        pending = list(setup_chunks(h + 1))
                    active.append(job_chunks(h, mb))
                    ji += 1
                for q_ in list(active):
                    q_.pop(0)()
                    drip += 1
                    if drip % 3 == 0 and pending:
                        pending.pop(0)()
                active = [q_ for q_ in active if q_]
            for c in pending:
                c()

    nc.compile()
    return nc


def _emit_pv(nc, ot, vaug, group, ntiles):
    pg, tiles = group
    for j, nt in enumerate(tiles):
        nc.tensor.matmul(
            ot[:],
            vaug[:, nt, :],
            pg[:, j, :],
            start=(nt == 0),
            stop=(nt == ntiles - 1),
            skip_group_check=True,
        )


_NC = None


def _get_nc():
    global _NC
    if _NC is None:
        _NC = build_nc()
    return _NC


def kernel(q: np.ndarray, k: np.ndarray, v: np.ndarray) -> np.ndarray:
    from concourse.bass_utils import run_bass_kernel_spmd

    nc = _get_nc()
    qf = np.ascontiguousarray(np.asarray(q, dtype=np.float32).reshape(B * H, N, D))
    kf = np.ascontiguousarray(np.asarray(k, dtype=np.float32).reshape(B * H, N, D))
    vf = np.ascontiguousarray(np.asarray(v, dtype=np.float32).reshape(B * H, N, D))
    in_maps = [
        {
            "q": np.ascontiguousarray(qf[c * HPC : (c + 1) * HPC]),
            "k": np.ascontiguousarray(kf[c * HPC : (c + 1) * HPC]),
            "v": np.ascontiguousarray(vf[c * HPC : (c + 1) * HPC]),
        }
        for c in range(NCORES)
    ]
    res = run_bass_kernel_spmd(nc, in_maps, core_ids=list(range(NCORES)))
    outs = [res.results[c]["out"] for c in range(NCORES)]
    return np.concatenate(outs, axis=0).reshape(B, H, N, D)


if __name__ == "__main__":
    rng = np.random.default_rng(0)
    qq = rng.standard_normal((B, H, N, D), dtype=np.float32)
    kk = rng.standard_normal((B, H, N, D), dtype=np.float32)
    vv = rng.standard_normal((B, H, N, D), dtype=np.float32)
    o = kernel(q=qq, k=kk, v=vv)
    print("kernel ran, out shape", o.shape, "finite:", np.isfinite(o).all())


# revision 35
# speedup vs baseline: 9902.6533x; 9352.7007x over previous
"""RBF-kernel causal attention on 8 Trainium2 NeuronCores.

B=2, H=16, N=2048, D=64. Shards the 32 (b,h) attention instances across 8
cores (4 heads per core). Math notes:

  logits = -relu(||q-k||^2)/sqrt(D); relu is a no-op (||q-k||^2 >= 0 up to
  rounding), and softmax is invariant to per-query offsets, so
      softmax_n(-(qsq_m + ksq_n - 2 qk)/8) == softmax_n(qk/4 - ksq_n/8)
  We compute P'' = exp(0.25 * K Q^T) in a [key, query] layout and fold the
  exp(-0.125 ksq_n) per-key factor into V (and into the appended ones-column
  that produces the softmax denominator):
      [O^T | l] accumulates via matmul(lhsT=V_aug_scaled, rhs=P'').
  Final output O[m,d] = OT[d,m] / l[m], un-transposed via PE transpose.

Emission is manually software-pipelined: head h+1's setup chunks (transposes,
ksq, V scaling) are interleaved between head h's query blocks so the tile
scheduler (limited lookahead) can overlap them.
"""

import sys

if "/opt/trn_rl_repo" not in sys.path:
    sys.path.insert(0, "/opt/trn_rl_repo")

import numpy as np

import concourse.bacc as bacc
import concourse.mybir as mybir
import concourse.tile as tile
from concourse.masks import make_identity

B, H, N, D = 2, 16, 2048, 64
NCORES = 8
HPC = (B * H) // NCORES  # heads per core = 4
P = 128                  # partitions
NT = N // P              # key tiles per head = 16
QB = 512                 # query block (matmul moving dim)
MBS = N // QB            # query blocks per head = 4
G = 2                    # key tiles per exp/ACT group (2 PSUM banks)

F32 = mybir.dt.float32
# float32r = relaxed-precision fp32 matmul (1 cycle/row at moving dim >= 256
# instead of 4 for float32)
MM_DT = mybir.dt.float32r


def build_nc():
    nc = bacc.Bacc("TRN2", target_bir_lowering=False, debug=False)
    q = nc.dram_tensor("q", [HPC, N, D], F32, kind="ExternalInput")
    k = nc.dram_tensor("k", [HPC, N, D], F32, kind="ExternalInput")
    v = nc.dram_tensor("v", [HPC, N, D], F32, kind="ExternalInput")
    out = nc.dram_tensor("out", [HPC, N, D], F32, kind="ExternalOutput")

    with tile.TileContext(nc) as tc:
        with (
            tc.tile_pool(name="const", bufs=1) as const_pool,
            tc.tile_pool(name="loads", bufs=1) as load_pool,
            tc.tile_pool(name="head", bufs=2) as head_pool,
            tc.tile_pool(name="work", bufs=3) as work_pool,
            tc.tile_pool(name="p", bufs=4) as p_pool,
            tc.tile_pool(name="epi", bufs=3) as epi_pool,
            tc.tile_pool(name="st", bufs=3, space="PSUM") as st_pool,
            tc.tile_pool(name="otp", bufs=2, space="PSUM") as ot_pool,
        ):
            identity = const_pool.tile([P, P], F32)
            make_identity(nc, identity)

            # prefetch every head's inputs up front: no-wait DMAs stream in
            # the background while compute proceeds
            knats, qnats, vtmps = [], [], []
            for h in range(HPC):
                # quarter-granular loads so the first transposes start as
                # soon as the first chunk lands, not after the whole head
                knat = load_pool.tile([P, NT, D], F32, tag=f"knat{h}")
                # q loaded DOUBLED along a repeat dim (two passes over DRAM):
                # transposing [128m, (2,64d)] then yields Q^T duplicated on
                # both partition halves, as the row-packed matmuls need
                qnat = load_pool.tile([P, NT, 2, D], F32, tag=f"qnat{h}")
                vtmp = load_pool.tile([P, NT, D], F32, tag=f"vtmp{h}")
                kq = k[h].rearrange("(t p) d -> p t d", p=P)
                qq = q[h].rearrange("(t p) d -> p t d", p=P)
                vq = v[h].rearrange("(t p) d -> p t d", p=P)
                nch = 4 if h == 0 else 1
                w_ = NT // nch
                for c in range(nch):
                    ts = slice(w_ * c, w_ * c + w_)
                    nc.sync.dma_start(knat[:, ts, :], kq[:, ts, :])
                    for r in range(2):
                        nc.sync.dma_start(qnat[:, ts, r, :], qq[:, ts, :])
                    nc.sync.dma_start(vtmp[:, ts, :], vq[:, ts, :])
                knats.append(knat)
                qnats.append(qnat)
                vtmps.append(vtmp)

            heads = [{} for _ in range(HPC)]

            def setup_chunks(h):
                """Emission chunks for head h's setup, in dependency order."""
                st = heads[h]

                def allocs():
                    st["ksq"] = head_pool.tile([P, NT], F32, tag="ksq", name="ksq")
                    st["w"] = head_pool.tile([P, NT], F32, tag="w", name="w")
                    st["vaug"] = head_pool.tile(
                        [P, NT, D + 1], MM_DT, tag="vaug", name="vaug"
                    )
                    # kt: key-tile PAIRS stacked on partition halves
                    # (even tile at partitions 0:64, odd at 64:128) so two
                    # QK matmuls can row-pack the PE array concurrently.
                    st["kt"] = head_pool.tile(
                        [P, NT // 2, P], MM_DT, tag="kt", name="kt"
                    )
                    # qt: Q^T duplicated into both partition halves (the
                    # row-packed matmuls stream rhs partitions 0:64 and
                    # 64:128 into array row groups 0-1 and 2-3)
                    st["qt"] = head_pool.tile([P, NT, P], MM_DT, tag="qt", name="qt")

                def scale_chunk(c):
                    # per-quarter V_aug build: runs as soon as that quarter
                    # of k and v has landed
                    def run():
                        ts = slice(4 * c, 4 * c + 4)
                        knat, vtmp = knats[h], vtmps[h]
                        ksq, w, vaug = st["ksq"], st["w"], st["vaug"]
                        ktmp = work_pool.tile([P, 4, D], F32, tag="ktmp")
                        nc.vector.tensor_mul(
                            out=ktmp[:], in0=knat[:, ts, :], in1=knat[:, ts, :]
                        )
                        nc.vector.tensor_reduce(
                            ksq[:, ts], ktmp[:],
                            axis=mybir.AxisListType.X, op=mybir.AluOpType.add,
                        )
                        nc.scalar.activation(
                            w[:, ts], ksq[:, ts],
                            mybir.ActivationFunctionType.Exp, scale=-0.125,
                        )
                        nc.vector.tensor_mul(
                            out=vaug[:, ts, :D],
                            in0=vtmp[:, ts, :],
                            in1=w[:, ts, None].to_broadcast((P, 4, D)),
                        )
                        nc.vector.tensor_copy(
                            out=vaug[:, ts, D : D + 1], in_=w[:, ts, None]
                        )

                    return run

                def ktr_group(g):
                    # 4 pair-transposes: [128n, (2t, 64d)] -> [(2t, 64d), 128n]
                    # lands even tile at partitions 0:64, odd at 64:128
                    def run():
                        src = knats[h]
                        dst = heads[h]["kt"]
                        tp = st_pool.tile([P, 4, P], F32, tag="stg", name="tp")
                        for j in range(4):
                            pr = 4 * g + j
                            nc.tensor.transpose(
                                tp[:, j, :], src[:, 2 * pr : 2 * pr + 2, :],
                                identity[:],
                            )
                        nc.vector.tensor_copy(
                            out=dst[:, 4 * g : 4 * g + 4, :], in_=tp[:]
                        )

                    return run

                def qtr_group(g):
                    # transpose a 0-stride doubled view [128m, (2, 64d)] so
                    # the output holds Q^T duplicated on both partition
                    # halves (rows 0:64 and 64:128) in one shot
                    def run():
                        src = qnats[h]
                        dst = heads[h]["qt"]
                        tp = st_pool.tile([P, 4, P], F32, tag="stg", name="tp")
                        for j in range(4):
                            nc.tensor.transpose(
                                tp[:, j, :], src[:, 4 * g + j, :, :], identity[:]
                            )
                        nc.vector.tensor_copy(
                            out=dst[:, 4 * g : 4 * g + 4, :], in_=tp[:]
                        )

                    return run

                # query block mb needs kt pair-groups up to (2mb+1)//4, qt
                # group mb, and vaug quarter mb; yield in dependency order
                yield allocs
                yield scale_chunk(0)
                yield ktr_group(0)
                yield qtr_group(0)
                yield scale_chunk(1)
                yield qtr_group(1)
                yield ktr_group(1)
                yield scale_chunk(2)
                yield qtr_group(2)
                yield scale_chunk(3)
                yield qtr_group(3)

            def job_chunks(h, mb):
                """Chunks of one (head, query-block) job, for interleaving."""
                kt, qt, vaug = heads[h]["kt"], heads[h]["qt"], heads[h]["vaug"]
                nsub = 4 * mb          # sub-diagonal key tiles
                qt_lo = qt[:D, 4 * mb : 4 * mb + 4, :]   # [64, 512]
                qt_hi = qt[D:, 4 * mb : 4 * mb + 4, :]   # [64, 512]
                ntiles = nsub + 4
                jst = {"prev": None, "ot": None}

                def sub_group(s):
                    def run():
                        if jst["ot"] is None:
                            jst["ot"] = ot_pool.tile(
                                [D + 1, QB], F32, tag="ot", name="ot"
                            )
                        stg = st_pool.tile([P, G, QB], F32, tag="stg")
                        pr = s // 2
                        nc.tensor.matmul(
                            stg[:, 0, :], kt[:D, pr, :], qt_lo,
                            start=True, stop=True, skip_group_check=True,
                        )
                        nc.tensor.matmul(
                            stg[:, 1, :], kt[D:, pr, :], qt_hi,
                            start=True, stop=True, skip_group_check=True,
                        )
                        pg = p_pool.tile([P, G, QB], MM_DT, tag="pg")
                        nc.scalar.activation(
                            pg[:], stg[:],
                            mybir.ActivationFunctionType.Exp, scale=0.25,
                        )
                        if jst["prev"] is not None:
                            _emit_pv(nc, jst["ot"], vaug, jst["prev"], ntiles)
                        jst["prev"] = (pg, [s, s + 1])

                    return run

                def diag_group(a):
                    def run():
                        if jst["ot"] is None:
                            jst["ot"] = ot_pool.tile(
                                [D + 1, QB], F32, tag="ot", name="ot"
                            )
                        if a == 0:
                            jst["pgd"] = p_pool.tile([P, 4, QB], MM_DT, tag="pgd", name="pgd")
                        pgd = jst["pgd"]
                        # columns m < 128*(2a) of tiles (2a, 2a+1) are fully
                        # masked: skip their QK matmul + exp; affine_select
                        # below zero-fills that (otherwise garbage) region.
                        c0 = P * 2 * a
                        stg = st_pool.tile([P, G, QB], F32, tag="stg")
                        pr = 2 * mb + a
                        nc.tensor.matmul(
                            stg[:, 0, c0:],
                            kt[:D, pr, :],
                            qt[:D, 4 * mb + 2 * a : 4 * mb + 4, :],
                            start=True, stop=True, skip_group_check=True,
                        )
                        nc.tensor.matmul(
                            stg[:, 1, c0:],
                            kt[D:, pr, :],
                            qt[D:, 4 * mb + 2 * a : 4 * mb + 4, :],
                            start=True, stop=True, skip_group_check=True,
                        )
                        nc.scalar.activation(
                            pgd[:, 2 * a : 2 * a + 2, c0:], stg[:, :, c0:],
                            mybir.ActivationFunctionType.Exp, scale=0.25,
                        )
                        for j in range(G):
                            # keep pgd[n, jj, m] iff m - n - 128 jj >= 0
                            jj = 2 * a + j
                            nc.gpsimd.affine_select(
                                out=pgd[:, jj, :], in_=pgd[:, jj, :],
                                compare_op=mybir.AluOpType.is_ge, fill=0.0,
                                base=-P * jj, pattern=[[1, QB]],
                                channel_multiplier=-1,
                            )

                    return run

                def pv_epilogue():
                    ot, pgd = jst["ot"], jst["pgd"]
                    if jst["prev"] is not None:
                        _emit_pv(nc, ot, vaug, jst["prev"], ntiles)
                    for j in range(4):
                        nc.tensor.matmul(
                            ot[:], vaug[:, 4 * mb + j, :], pgd[:, j, :],
                            start=(nsub == 0 and j == 0), stop=(j == 3),
                            skip_group_check=True,
                        )
                    # epilogue: transpose + normalize + store
                    ot_sb = epi_pool.tile([D + 1, QB], F32, tag="ot_sb")
                    nc.vector.tensor_copy(out=ot_sb[:], in_=ot[:])
                    tpo = ot_pool.tile([P, 4, D + 1], F32, tag="ot", name="tpo")
                    for j in range(4):
                        nc.tensor.transpose(
                            tpo[:, j, :],
                            ot_sb[:, j * P : (j + 1) * P],
                            identity[: D + 1, : D + 1],
                        )
                    linv = epi_pool.tile([P, 4], F32, tag="linv")
                    nc.vector.reciprocal(linv[:], tpo[:, :, D])
                    o_sb = epi_pool.tile([P, 4, D], F32, tag="o_sb")
                    for j in range(4):
                        nc.vector.tensor_scalar_mul(
                            o_sb[:, j, :], tpo[:, j, :D], linv[:, j : j + 1]
                        )
                    nc.sync.dma_start(
                        out[h, mb * QB : (mb + 1) * QB, :].rearrange(
                            "(j p) d -> p j d", p=P
                        ),
                        o_sb[:],
                    )

                chunks = [sub_group(s) for s in range(0, nsub, G)]
                chunks += [diag_group(0), diag_group(1), pv_epilogue]
                return chunks

            # ---- software-pipelined emission: depth-2 job interleave ----
            for c in setup_chunks(0):
                c()
            pending = []          # next head's setup chunks, dripped in
            jobs = [(h, mb) for h in range(HPC) for mb in range(MBS)]
            active = []           # up to 2 jobs' chunk queues
            ji = 0
            drip = 0
            while active or ji < len(jobs):
                while len(active) < 2 and ji < len(jobs):
                    h, mb = jobs[ji]
                    if mb == 0 and pending:
                        # head h's setup must be fully emitted before its
                        # first job
                        for c in pending:
                            c()
                        pending = []
                    if mb == 0 and h + 1 < HPC:
                        pending = list(setup_chunks(h + 1))
                    active.append(job_chunks(h, mb))
                    ji += 1
                for q_ in list(active):
                    q_.pop(0)()
                    drip += 1
                    if drip % 3 == 0 and pending:
                        pending.pop(0)()
                active = [q_ for q_ in active if q_]
            for c in pending:
                c()

    nc.compile()
    return nc


def _emit_pv(nc, ot, vaug, group, ntiles):
    pg, tiles = group
    for j, nt in enumerate(tiles):
        nc.tensor.matmul(
            ot[:],
            vaug[:, nt, :],
            pg[:, j, :],
            start=(nt == 0),
            stop=(nt == ntiles - 1),
            skip_group_check=True,
        )


_NC = None


def _get_nc():
    global _NC
    if _NC is None:
        _NC = build_nc()
    return _NC


def kernel(q: np.ndarray, k: np.ndarray, v: np.ndarray) -> np.ndarray:
    from concourse.bass_utils import run_bass_kernel_spmd

    nc = _get_nc()
    qf = np.ascontiguousarray(np.asarray(q, dtype=np.float32).reshape(B * H, N, D))
    kf = np.ascontiguousarray(np.asarray(k, dtype=np.float32).reshape(B * H, N, D))
    vf = np.ascontiguousarray(np.asarray(v, dtype=np.float32).reshape(B * H, N, D))
    in_maps = [
        {
            "q": np.ascontiguousarray(qf[c * HPC : (c + 1) * HPC]),
            "k": np.ascontiguousarray(kf[c * HPC : (c + 1) * HPC]),
            "v": np.ascontiguousarray(vf[c * HPC : (c + 1) * HPC]),
        }
        for c in range(NCORES)
    ]
    res = run_bass_kernel_spmd(nc, in_maps, core_ids=list(range(NCORES)))
    outs = [res.results[c]["out"] for c in range(NCORES)]
    return np.concatenate(outs, axis=0).reshape(B, H, N, D)


if __name__ == "__main__":
    rng = np.random.default_rng(0)
    qq = rng.standard_normal((B, H, N, D), dtype=np.float32)
    kk = rng.standard_normal((B, H, N, D), dtype=np.float32)
    vv = rng.standard_normal((B, H, N, D), dtype=np.float32)
    o = kernel(q=qq, k=kk, v=vv)
    print("kernel ran, out shape", o.shape, "finite:", np.isfinite(o).all())
